# revision 3
# baseline (speedup 1.0000x reference)
"""Trainium2 Bass kernel for nn_Pool_80135499809385 (topk_masking).

Reference computation (per batch b of B=4):
    scores = sigmoid(h[b] @ section_feature[b,0])          # [N]
    values, idx = top_k(scores, K)                         # [K]
    g_section  = g1[b][idx][:, idx]                        # [K, K]
    g_sentence = g2[b][idx][:, idx]                        # [K, K]
    g_mask     = g3[b][idx]                                # [K, N]
    new_h      = h[b][idx] * values[:, None]               # [K, D]

Sharding: 8 cores = 4 batches x 2 row-halves (data parallel, no comms).
Each core gathers its 1024 output rows. The column gather for g1/g2 is done
as: row-gather -> PE transpose -> indirect-DMA scatter (with OOB-skip
compaction/permutation into idx order) into a DRAM scratch C[K, 1024] ->
contiguous readback -> PE transpose -> row-major output.

The tiny score/top-k computation runs via jax on a NeuronCore (bitwise
identical to the reference); its idx/values feed the Bass kernel as small
int32/f32 side inputs driving runtime-indirect DMAs.
"""

import sys

import numpy as np

for _p in ("/opt/trn_rl_repo",):
    if _p not in sys.path:
        sys.path.append(_p)

B = 4
N = 4096
D = 512
K = 2048
P = 128
HALF = K // 2          # rows per core (1024)
NT = HALF // P         # row chunks per core (8)
NJ = N // P            # j-blocks (32)
OOB = 1 << 20          # scatter offset sentinel: skipped via bounds_check

_PROGRAM_CACHE = {}


def _build_program():
    from contextlib import ExitStack

    import concourse.bass as bass
    import concourse.tile as tile
    from concourse import bacc, mybir
    from concourse.masks import make_identity

    f32 = mybir.dt.float32
    i32 = mybir.dt.int32

    nc = bacc.Bacc(
        "TRN2", target_bir_lowering=False, debug=False, num_devices=8
    )

    g1 = nc.dram_tensor("g1", [N, N], f32, kind="ExternalInput").ap()
    g2 = nc.dram_tensor("g2", [N, N], f32, kind="ExternalInput").ap()
    g3 = nc.dram_tensor("g3", [N, N], f32, kind="ExternalInput").ap()
    h = nc.dram_tensor("h", [N, D], f32, kind="ExternalInput").ap()
    rowsel = nc.dram_tensor("rowsel", [P, NT], i32, kind="ExternalInput").ap()
    scatoff = nc.dram_tensor("scatoff", [P, NJ], i32, kind="ExternalInput").ap()
    vals = nc.dram_tensor("vals", [P, NT], f32, kind="ExternalInput").ap()

    gsec = nc.dram_tensor("gsec", [HALF, K], f32, kind="ExternalOutput").ap()
    gsen = nc.dram_tensor("gsen", [HALF, K], f32, kind="ExternalOutput").ap()
    gmask = nc.dram_tensor("gmask", [HALF, N], f32, kind="ExternalOutput").ap()
    newh = nc.dram_tensor("newh", [HALF, D], f32, kind="ExternalOutput").ap()

    with tile.TileContext(nc) as tc, ExitStack() as ctx:
        const_pool = ctx.enter_context(tc.tile_pool(name="const", bufs=1))
        g_pool = ctx.enter_context(tc.tile_pool(name="gsub", bufs=5))
        gt_pool = ctx.enter_context(tc.tile_pool(name="gt", bufs=3))
        c_pool = ctx.enter_context(tc.tile_pool(name="ct", bufs=3))
        out_pool = ctx.enter_context(tc.tile_pool(name="outt", bufs=1))
        stream_pool = ctx.enter_context(tc.tile_pool(name="stream", bufs=3))
        psum_pool = ctx.enter_context(
            tc.tile_pool(name="psum", bufs=4, space="PSUM")
        )
        dram_pool = ctx.enter_context(
            tc.tile_pool(name="dram", bufs=1, space="DRAM")
        )

        identity = const_pool.tile([P, P], f32, name="identity")
        make_identity(nc, identity[:])
        rowsel_sb = const_pool.tile([P, NT], i32, name="rowsel_sb")
        nc.sync.dma_start(rowsel_sb[:], rowsel)
        scatoff_sb = const_pool.tile([P, NJ], i32, name="scatoff_sb")
        nc.sync.dma_start(scatoff_sb[:], scatoff)
        vals_sb = const_pool.tile([P, NT], f32, name="vals_sb")
        nc.sync.dma_start(vals_sb[:], vals)

        def emit_graph(gsrc, gout, cname):
            cdram = dram_pool.tile([K, HALF], f32, name=cname)
            # Phase A: row gather (128 rows x 16KiB per call)
            gsubs = []
            for t in range(NT):
                gs = g_pool.tile([P, N], f32, tag="gsub", name=f"{cname}_g{t}")
                nc.gpsimd.indirect_dma_start(
                    out=gs[:],
                    out_offset=None,
                    in_=gsrc,
                    in_offset=bass.IndirectOffsetOnAxis(
                        ap=rowsel_sb[:, t : t + 1], axis=0
                    ),
                )
                gsubs.append(gs)
            # Phase B: transpose 512-row chunks, scatter-compact into C
            for q in range(2):
                for jp in range(NJ // 2):
                    ps = psum_pool.tile([P, 1024], f32, tag="ps")
                    for j2 in range(2):
                        jblk = jp * 2 + j2
                        for tp in range(4):
                            nc.tensor.transpose(
                                out=ps[
                                    :,
                                    j2 * 512 + tp * P : j2 * 512 + (tp + 1) * P,
                                ],
                                in_=gsubs[q * 4 + tp][
                                    :, jblk * P : (jblk + 1) * P
                                ],
                                identity=identity[:],
                            )
                    gt = gt_pool.tile([P, 1024], f32, tag="gt")
                    nc.vector.tensor_copy(gt[:], ps[:])
                    # HW indirect DMA: one offset per partition -> per-J calls
                    for j2 in range(2):
                        jblk = jp * 2 + j2
                        nc.gpsimd.indirect_dma_start(
                            out=cdram[:],
                            out_offset=bass.IndirectOffsetOnAxis(
                                ap=scatoff_sb[:, jblk : jblk + 1], axis=0
                            ),
                            in_=gt[:, j2 * 512 : (j2 + 1) * 512],
                            in_offset=None,
                            element_offset=q * 512,
                            bounds_check=K - 1,
                            oob_is_err=False,
                        )
            # Phase C: contiguous readback, transpose to row-major output
            for half in range(2):
                outt = out_pool.tile([P, NT * 1024], f32, tag="outt")
                outt3 = outt[:].rearrange("p (t m) -> p t m", t=NT)
                for u8 in range(8):
                    u = half * 8 + u8
                    ct = c_pool.tile([P, HALF], f32, tag="ct")
                    nc.sync.dma_start(
                        out=ct[:], in_=cdram[u * P : (u + 1) * P, :]
                    )
                    ps = psum_pool.tile([P, 1024], f32, tag="ps")
                    for t in range(NT):
                        nc.tensor.transpose(
                            out=ps[:, t * P : (t + 1) * P],
                            in_=ct[:, t * P : (t + 1) * P],
                            identity=identity[:],
                        )
                    nc.vector.tensor_copy(
                        outt3[:, :, u8 * P : (u8 + 1) * P],
                        ps[:].rearrange("p (t m) -> p t m", t=NT),
                    )
                for t in range(NT):
                    nc.sync.dma_start(
                        out=gout[
                            t * P : (t + 1) * P,
                            half * 1024 : (half + 1) * 1024,
                        ],
                        in_=outt3[:, t, :],
                    )

        emit_graph(g1, gsec, "c1")
        emit_graph(g2, gsen, "c2")

        # g_mask: pure row gather + copy out
        for t in range(NT):
            g3t = stream_pool.tile([P, N], f32, tag="stream", name=f"g3_{t}")
            nc.gpsimd.indirect_dma_start(
                out=g3t[:],
                out_offset=None,
                in_=g3,
                in_offset=bass.IndirectOffsetOnAxis(
                    ap=rowsel_sb[:, t : t + 1], axis=0
                ),
            )
            nc.sync.dma_start(out=gmask[t * P : (t + 1) * P, :], in_=g3t[:])

        # new_h: row gather (one offset per partition per call) + scale
        ht = stream_pool.tile([P, NT * D], f32, tag="stream", name="ht")
        for t in range(NT):
            nc.gpsimd.indirect_dma_start(
                out=ht[:, t * D : (t + 1) * D],
                out_offset=None,
                in_=h,
                in_offset=bass.IndirectOffsetOnAxis(
                    ap=rowsel_sb[:, t : t + 1], axis=0
                ),
            )
        for t in range(NT):
            nc.vector.tensor_scalar_mul(
                ht[:, t * D : (t + 1) * D],
                ht[:, t * D : (t + 1) * D],
                vals_sb[:, t : t + 1],
            )
        nc.sync.dma_start(
            out=newh.rearrange("(t p) d -> p t d", p=P),
            in_=ht[:].rearrange("p (t d) -> p t d", t=NT),
        )

    nc.compile()
    return nc


def get_program():
    if "nc" not in _PROGRAM_CACHE:
        _PROGRAM_CACHE["nc"] = _build_program()
    return _PROGRAM_CACHE["nc"]


def topk_host(h, section_feature):
    """scores + top-k, bitwise-matching the reference (same jax ops)."""
    import jax
    import jax.numpy as jnp

    scores = jax.nn.sigmoid(
        jnp.einsum(
            "bnd,bd->bn",
            jnp.asarray(h),
            jnp.asarray(section_feature)[:, 0, :],
        )
    )
    values, idx = jax.lax.top_k(scores, K)
    return np.asarray(values).astype(np.float32), np.asarray(idx)


def make_in_maps(g1, g2, g3, h, values, idx):
    in_maps = []
    for core in range(8):
        b, hh = divmod(core, 2)
        sel = idx[b, hh * HALF : (hh + 1) * HALF].astype(np.int64)
        rowsel = np.ascontiguousarray(
            sel.reshape(NT, P).T.astype(np.int32)
        )  # [p, t] = sel[t*128+p]
        vv = np.ascontiguousarray(
            values[b, hh * HALF : (hh + 1) * HALF]
            .reshape(NT, P)
            .T.astype(np.float32)
        )
        inv = np.full(N, OOB, np.int64)
        inv[idx[b].astype(np.int64)] = np.arange(K)
        scat = np.ascontiguousarray(
            inv.reshape(NJ, P).T.astype(np.int32)
        )  # [p, J] = inv[J*128+p]
        in_maps.append(
            {
                "g1": np.ascontiguousarray(g1[b]),
                "g2": np.ascontiguousarray(g2[b]),
                "g3": np.ascontiguousarray(g3[b]),
                "h": np.ascontiguousarray(h[b]),
                "rowsel": rowsel,
                "scatoff": scat,
                "vals": vv,
            }
        )
    return in_maps


def assemble(results):
    def cat(name):
        return np.stack(
            [
                np.concatenate(
                    [results[2 * b][name], results[2 * b + 1][name]], axis=0
                )
                for b in range(B)
            ]
        )

    return cat("gsec"), cat("gsen"), cat("gmask"), cat("newh")


def kernel(g1, g2, g3, h, section_feature):
    from concourse.bass_utils import run_bass_kernel_spmd

    g1 = np.asarray(g1, dtype=np.float32)
    g2 = np.asarray(g2, dtype=np.float32)
    g3 = np.asarray(g3, dtype=np.float32)
    h = np.asarray(h, dtype=np.float32)
    section_feature = np.asarray(section_feature, dtype=np.float32)

    values, idx = topk_host(h, section_feature)
    nc = get_program()
    in_maps = make_in_maps(g1, g2, g3, h, values, idx)
    res = run_bass_kernel_spmd(nc, in_maps, list(range(8))).results
    return assemble(res)
